# revision 1
# baseline (speedup 1.0000x reference)
"""Trainium2 Bass kernel for nn_Cross_Attention (8-core data-parallel over batch).

Reference computation per batch item:
  kvf  = conv1x1(kv, qkv1_w)                    # [384, H, W]
  kvd  = depthwise3x3(kvf, qkv2_w, pad=1)       # [384, H, W]
  k, v = split(kvd); qh/kh L2-normalized over hw per head-channel row
  attn = softmax(scale * qn @ kn^T)             # [8, 24, 24] block per head
  out  = proj1x1(attn @ v, proj_w)              # [192, H, W]

Each NeuronCore processes one batch item end-to-end (no collectives).
SBUF pressure is managed with one bufs=1 pool whose tags alias big tensors
with disjoint lifetimes (q16 reuses k16's slots, kB reuses kvf's slot).
"""

import os
import sys

sys.path.insert(0, "/opt/trn_rl_repo")

import numpy as np

import concourse.bass as bass
import concourse.tile as tile
from concourse import bacc, mybir
from concourse.bass_utils import run_bass_kernel_spmd
from concourse.bass_interp import get_hw_module

F32 = mybir.dt.float32
F16 = mybir.dt.float16

# Problem dims (per core / batch item)
C = 192          # input channels
C2 = 384         # conv1 output channels
HEADS = 8
CD = C // HEADS  # 24 channels per head
W = 128          # image cols (== partition width for pixel tiles)
H = int(os.environ.get("BASS_CA_H", "128"))  # image rows (overridable for sim)
HWTOT = H * W
PT = 512                    # pixels per matmul tile (one PSUM bank fp32)
RPT = PT // W               # image rows per tile (4)
NT = HWTOT // PT            # pixel tiles (32 at H=128)
PADR = 2                    # zero-pad rows on each side of kvf (covers dr=+-1)
EPS = 1e-12

# taps: (dr, dc), weight index = (dr+1)*3 + (dc+1); (0,0) first so the
# first matmul of each PSUM accumulation group writes every element.
TAPS = [(0, 0)] + [(dr, dc) for dr in (-1, 0, 1) for dc in (-1, 0, 1)
                   if not (dr == 0 and dc == 0)]


def sl(nt, size=PT):
    return slice(nt * size, (nt + 1) * size)


def emit_kernel(tc, io):
    nc = tc.nc
    kv, q, w1t, w2d, wpt, ident, mask, scale192 = (
        io["kv"], io["q"], io["w1t"], io["w2d"], io["wpt"], io["ident"],
        io["mask"], io["scale192"])
    out = io["out"]
    vdram = io["vdram"]
    kv16d = io["kv16d"]

    from contextlib import ExitStack
    _stack = ExitStack()
    wp = _stack.enter_context(tc.tile_pool(name="weights", bufs=1))
    sml = _stack.enter_context(tc.tile_pool(name="small", bufs=1))
    big = _stack.enter_context(tc.tile_pool(name="big", bufs=1))

    # ---- weights to SBUF ----
    w1ta = wp.tile([128, C2], F16); nc.sync.dma_start(w1ta[:], w1t[0:128, :])
    w1tb = wp.tile([64, C2], F16); nc.sync.dma_start(w1tb[:], w1t[128:C, :])
    wpta = wp.tile([128, C], F16); nc.sync.dma_start(wpta[:], wpt[0:128, :])
    wptb = wp.tile([64, C], F16); nc.sync.dma_start(wptb[:], wpt[128:C, :])
    id16 = wp.tile([128, 128], F16); nc.sync.dma_start(id16[:], ident[:])
    maska = wp.tile([128, C], F32); nc.sync.dma_start(maska[:], mask[0:128, :])
    maskb = wp.tile([64, C], F32); nc.sync.dma_start(maskb[:], mask[128:C, :])
    sca = wp.tile([128, 1], F32); nc.sync.dma_start(sca[:], scale192[0:128, :])
    scb = wp.tile([64, 1], F32); nc.sync.dma_start(scb[:], scale192[128:C, :])
    w2sb = wp.tile([128, 27, 128], F16)
    nc.sync.dma_start(w2sb[:], w2d.rearrange("t p c -> p t c"))

    spa = sml.tile([128, 1], F32)
    spb = sml.tile([64, 1], F32)
    bda = sml.tile([128, C], F16)
    bdb = sml.tile([64, C], F16)

    # ---- stage 0: kv f32 -> f16 copy in DRAM (conv1 re-reads it 3x) ----
    with tc.tile_pool(name="cvt", bufs=3) as cst:
        for nt in range(NT):
            sa = cst.tile([128, PT], F32, tag="sa")
            nc.sync.dma_start(sa[:], kv[0:128, sl(nt)])
            fa = cst.tile([128, PT], F16, tag="fa")
            nc.any.tensor_copy(fa[:], sa[:])
            nc.sync.dma_start(kv16d[0:128, sl(nt)], fa[:])
            sb = cst.tile([64, PT], F32, tag="sb")
            nc.sync.dma_start(sb[:], kv[128:C, sl(nt)])
            fb = cst.tile([64, PT], F16, tag="fb")
            nc.any.tensor_copy(fb[:], sb[:])
            nc.sync.dma_start(kv16d[128:C, sl(nt)], fb[:])

    # ================= conv1 + depthwise =================
    k16a = big.tile([128, HWTOT], F16, tag="slot_ka")
    k16b = big.tile([64, HWTOT], F16, tag="slot_kb")

    with tc.tile_pool(name="convst", bufs=3) as st, \
         tc.tile_pool(name="psA", bufs=2, space="PSUM") as psA, \
         tc.tile_pool(name="psB", bufs=1, space="PSUM") as psB, \
         tc.tile_pool(name="vstage", bufs=2) as vst:
        for mc in range(3):
            kvf = big.tile([128, (H + 2 * PADR) * W], F16, tag="slot_kvf",
                           name="kvf")
            kvf3 = kvf[:].rearrange("p (r c) -> p r c", c=W)
            nc.vector.memset(kvf3[:, 0:PADR, :], 0.0)
            nc.vector.memset(kvf3[:, PADR + H:, :], 0.0)
            # conv1: kvf[mc*128 + ch, pix] = sum_cin w1[ch, cin] kv[cin, pix]
            for nt in range(NT):
                ka = st.tile([128, PT], F16, tag="ka")
                nc.sync.dma_start(ka[:], kv16d[0:128, sl(nt)])
                kb = st.tile([64, PT], F16, tag="kb")
                nc.sync.dma_start(kb[:], kv16d[128:C, sl(nt)])
                ps = psA.tile([128, PT], F32, tag="psA")
                nc.tensor.matmul(ps[:], w1ta[:, mc * 128:(mc + 1) * 128],
                                 ka[:], start=True, stop=False)
                nc.tensor.matmul(ps[:], w1tb[:, mc * 128:(mc + 1) * 128],
                                 kb[:], start=False, stop=True)
                nc.any.tensor_copy(kvf[:, PADR * W + nt * PT:
                                       PADR * W + (nt + 1) * PT], ps[:])
            # depthwise 3x3 via diagonal-weight matmuls, accumulate in PSUM.
            # PSUM pixel tiles are col-major ([c*RPT + r]) so dc-shifted
            # output slices stay flat-contiguous (1 free dim).
            kvfT3 = kvf[:].rearrange("p (r c) -> p c r", c=W)
            for g in range(0, NT, 6):
                gn = min(6, NT - g)
                pss = [psB.tile([128, PT], F32, tag=f"psB{j}",
                                name=f"psB{j}") for j in range(gn)]
                for ti, (dr, dc) in enumerate(TAPS):
                    wi = (dr + 1) * 3 + (dc + 1)
                    lw = w2sb[:, mc * 9 + wi, :]
                    if dc == 0:
                        ci, co = slice(0, W), slice(0, PT)
                    elif dc == -1:
                        ci, co = slice(0, W - 1), slice(RPT, PT)
                    else:
                        ci, co = slice(1, W), slice(0, PT - RPT)
                    for j in range(gn):
                        r0 = (g + j) * RPT
                        rs = slice(PADR + r0 + dr, PADR + r0 + dr + RPT)
                        nc.tensor.matmul(pss[j][:, co], lw, kvfT3[:, ci, rs],
                                         start=(ti == 0), stop=(ti == 8))
                # evacuate to k (ch < 192) and v (ch >= 192); destination
                # views are col-major to match the PSUM layout
                for j in range(gn):
                    nt = g + j
                    pcm = pss[j][:]

                    def cmv(ap):
                        return ap.rearrange("p (r c) -> p c r", r=RPT)
                    if mc == 0:
                        nc.any.tensor_copy(cmv(k16a[:, sl(nt)]), pcm)
                    elif mc == 1:
                        nc.any.tensor_copy(cmv(k16b[:, sl(nt)]), pcm[0:64, :])
                        vs = vst.tile([128, PT], F16, tag="vs")
                        nc.any.tensor_copy(cmv(vs[64:128, :]), pcm[64:128, :])
                        nc.sync.dma_start(vdram[0:64, sl(nt)], vs[64:128, :])
                    else:
                        vs = vst.tile([128, PT], F16, tag="vs")
                        nc.any.tensor_copy(cmv(vs[:]), pcm)
                        nc.sync.dma_start(vdram[64:C, sl(nt)], vs[:])

    # ================= k norms, scale, transpose =================
    with tc.tile_pool(name="norm", bufs=1) as npl:
        nk2a = npl.tile([128, 1], F32)
        nk2b = npl.tile([64, 1], F32)
        NCH = 8
        CHW = HWTOT // NCH
        kparts_a = npl.tile([128, NCH], F32)
        kparts_b = npl.tile([64, NCH], F32)
        sqs = npl.tile([128, CHW], F16)
        for i in range(NCH):
            nc.scalar.activation(sqs[:, :], k16a[:, sl(i, CHW)],
                                 mybir.ActivationFunctionType.Square,
                                 accum_out=kparts_a[:, i:i + 1])
        for i in range(NCH):
            nc.scalar.activation(sqs[0:64, :], k16b[:, sl(i, CHW)],
                                 mybir.ActivationFunctionType.Square,
                                 accum_out=kparts_b[:, i:i + 1])
        nc.vector.reduce_sum(nk2a[:], kparts_a[:], axis=mybir.AxisListType.X)
        nc.vector.reduce_sum(nk2b[:], kparts_b[:], axis=mybir.AxisListType.X)
        for nk2 in (nk2a, nk2b):
            nc.scalar.sqrt(nk2[:], nk2[:])
            nc.vector.tensor_scalar_max(nk2[:], nk2[:], EPS)
            nc.vector.reciprocal(nk2[:], nk2[:])
        nc.vector.tensor_scalar_mul(k16a[:], k16a[:], nk2a[:])
        nc.vector.tensor_scalar_mul(k16b[:], k16b[:], nk2b[:])

        # kBa reuses kvf's slot (kvf is dead after the depthwise)
        kBa = big.tile([128, H, 128], F16, tag="slot_kvf", name="kBa")
        kBb = big.tile([128, H, 64], F16, tag="slot_kbb", name="kBb")
        nc.sync.dma_start_transpose(kBa[:], k16a[:])
        nc.sync.dma_start_transpose(kBb[:], k16b[:])

        # ========== q: load, norms (q16 reuses k16 slots) ==========
        q16a = big.tile([128, HWTOT], F16, tag="slot_ka", name="q16a")
        q16b = big.tile([64, HWTOT], F16, tag="slot_kb", name="q16b")
        qpa = npl.tile([128, NT], F32)
        qpb = npl.tile([64, NT], F32)
        with tc.tile_pool(name="qstage", bufs=3) as qst:
            for nt in range(NT):
                sa = qst.tile([128, PT], F32, tag="qsa")
                nc.sync.dma_start(sa[:], q[0:128, sl(nt)])
                nc.any.tensor_copy(q16a[:, sl(nt)], sa[:])
                qsq = qst.tile([128, PT], F16, tag="qsq")
                nc.scalar.activation(qsq[:], q16a[:, sl(nt)],
                                     mybir.ActivationFunctionType.Square,
                                     accum_out=qpa[:, nt:nt + 1])
                sb = qst.tile([64, PT], F32, tag="qsb")
                nc.sync.dma_start(sb[:], q[128:C, sl(nt)])
                nc.any.tensor_copy(q16b[:, sl(nt)], sb[:])
                nc.scalar.activation(qsq[0:64, :], q16b[:, sl(nt)],
                                     mybir.ActivationFunctionType.Square,
                                     accum_out=qpb[:, nt:nt + 1])
        nq2a = npl.tile([128, 1], F32)
        nq2b = npl.tile([64, 1], F32)
        nc.vector.reduce_sum(nq2a[:], qpa[:], axis=mybir.AxisListType.X)
        nc.vector.reduce_sum(nq2b[:], qpb[:], axis=mybir.AxisListType.X)
        for nq2, spx, scx in ((nq2a, spa, sca), (nq2b, spb, scb)):
            nc.scalar.sqrt(nq2[:], nq2[:])
            nc.vector.tensor_scalar_max(nq2[:], nq2[:], EPS)
            nc.vector.reciprocal(nq2[:], nq2[:])
            nc.vector.tensor_tensor(out=spx[:], in0=nq2[:], in1=scx[:],
                                    op=mybir.AluOpType.mult)

        qBa = big.tile([128, H, 128], F16, tag="slot_qba", name="qBa")
        qBb = big.tile([128, H, 64], F16, tag="slot_qbb", name="qBb")
        nc.sync.dma_start_transpose(qBa[:], q16a[:])
        nc.sync.dma_start_transpose(qBb[:], q16b[:])

    # ================= Gram =================
    with tc.tile_pool(name="psG", bufs=1, space="PSUM") as psG, \
         tc.tile_pool(name="smax", bufs=1) as sm, \
         tc.tile_pool(name="psT", bufs=1, space="PSUM") as psT:
        G0a = psG.tile([128, 128], F32, tag="G0a", name="G0a")
        G0b = psG.tile([128, 64], F32, tag="G0b", name="G0b")
        G1a = psG.tile([64, 128], F32, tag="G1a", name="G1a")
        G1b = psG.tile([64, 64], F32, tag="G1b", name="G1b")
        for t in range(H):
            s0, s1 = (t == 0), (t == H - 1)
            nc.tensor.matmul(G0a[:], qBa[:, t, :], kBa[:, t, :],
                             start=s0, stop=s1)
            nc.tensor.matmul(G0b[:], qBa[:, t, :], kBb[:, t, :],
                             start=s0, stop=s1)
            nc.tensor.matmul(G1a[:], qBb[:, t, :], kBa[:, t, :],
                             start=s0, stop=s1)
            nc.tensor.matmul(G1b[:], qBb[:, t, :], kBb[:, t, :],
                             start=s0, stop=s1)

        # ---- masked softmax over d (free dim), build block-diag attn^T ----
        for Ga, Gb, spx, mkx, rows in ((G0a, G0b, spa, maska, 128),
                                       (G1a, G1b, spb, maskb, 64)):
            lg = sm.tile([rows, C], F32, tag=f"lg{rows}", name=f"lg{rows}")
            nc.vector.scalar_tensor_tensor(
                out=lg[:, 0:128], in0=Ga[:], scalar=spx[:], in1=mkx[:, 0:128],
                op0=mybir.AluOpType.mult, op1=mybir.AluOpType.add)
            nc.vector.scalar_tensor_tensor(
                out=lg[:, 128:C], in0=Gb[:], scalar=spx[:], in1=mkx[:, 128:C],
                op0=mybir.AluOpType.mult, op1=mybir.AluOpType.add)
            mx = sm.tile([rows, 1], F32, tag=f"mx{rows}", name=f"mx{rows}")
            nc.vector.reduce_max(mx[:], lg[:], axis=mybir.AxisListType.X)
            nc.vector.tensor_scalar_mul(mx[:], mx[:], -1.0)
            ssum = sm.tile([rows, 1], F32, tag=f"ss{rows}", name=f"ss{rows}")
            nc.scalar.activation(lg[:], lg[:], mybir.ActivationFunctionType.Exp,
                                 bias=mx[:], accum_out=ssum[:])
            nc.vector.reciprocal(ssum[:], ssum[:])
            at16 = sm.tile([rows, C], F16, tag=f"at{rows}", name=f"at{rows}")
            nc.vector.tensor_scalar_mul(at16[:], lg[:], ssum[:])
            # transpose [rows, C] attn block into BD tiles
            tp0 = psT.tile([128, 128], F16, tag="tp0", name="tp0")
            nc.tensor.transpose(tp0[0:128, 0:rows], at16[:, 0:128],
                                id16[0:rows, 0:rows])
            tp1 = psT.tile([128, 128], F16, tag="tp1", name="tp1")
            nc.tensor.transpose(tp1[0:64, 0:rows], at16[:, 128:C],
                                id16[0:rows, 0:rows])
            if rows == 128:
                nc.any.tensor_copy(bda[:, 0:128], tp0[0:128, 0:128])
                nc.any.tensor_copy(bdb[:, 0:128], tp1[0:64, 0:128])
            else:
                nc.any.tensor_copy(bda[:, 128:C], tp0[0:128, 0:64])
                nc.any.tensor_copy(bdb[:, 128:C], tp1[0:64, 0:64])

    # ================= O = attn @ v, then proj =================
    with tc.tile_pool(name="ostage", bufs=3) as ost, \
         tc.tile_pool(name="psO", bufs=1, space="PSUM") as psO:
        for nt in range(NT):
            va = ost.tile([128, PT], F16, tag="va")
            nc.sync.dma_start(va[:], vdram[0:128, sl(nt)])
            vb = ost.tile([64, PT], F16, tag="vb")
            nc.sync.dma_start(vb[:], vdram[128:C, sl(nt)])
            O0 = psO.tile([128, PT], F32, tag="O0")
            O1 = psO.tile([64, PT], F32, tag="O1")
            nc.tensor.matmul(O0[:], bda[:, 0:128], va[:], start=True, stop=False)
            nc.tensor.matmul(O0[:], bdb[:, 0:128], vb[:], start=False, stop=True)
            nc.tensor.matmul(O1[:], bda[:, 128:C], va[:], start=True, stop=False)
            nc.tensor.matmul(O1[:], bdb[:, 128:C], vb[:], start=False, stop=True)
            oa = ost.tile([128, PT], F16, tag="oa")
            ob = ost.tile([64, PT], F16, tag="ob")
            nc.any.tensor_copy(oa[:], O0[:])
            nc.any.tensor_copy(ob[:], O1[:])
            P0 = psO.tile([128, PT], F32, tag="P0")
            P1 = psO.tile([64, PT], F32, tag="P1")
            nc.tensor.matmul(P0[:], wpta[:, 0:128], oa[:], start=True, stop=False)
            nc.tensor.matmul(P0[:], wptb[:, 0:128], ob[:], start=False, stop=True)
            nc.tensor.matmul(P1[:], wpta[:, 128:C], oa[:], start=True, stop=False)
            nc.tensor.matmul(P1[:], wptb[:, 128:C], ob[:], start=False, stop=True)
            fa = ost.tile([128, PT], F32, tag="fa")
            fb = ost.tile([64, PT], F32, tag="fb")
            nc.any.tensor_copy(fa[:], P0[:])
            nc.any.tensor_copy(fb[:], P1[:])
            nc.sync.dma_start(out[0:128, sl(nt)], fa[:])
            nc.sync.dma_start(out[128:C, sl(nt)], fb[:])
    _stack.close()


def build_module():
    nc = bacc.Bacc("TRN2")
    io = {}
    io["kv"] = nc.dram_tensor("kv", [C, HWTOT], F32, kind="ExternalInput").ap()
    io["q"] = nc.dram_tensor("q", [C, HWTOT], F32, kind="ExternalInput").ap()
    io["w1t"] = nc.dram_tensor("w1t", [C, C2], F16, kind="ExternalInput").ap()
    io["w2d"] = nc.dram_tensor("w2d", [27, 128, 128], F16, kind="ExternalInput").ap()
    io["wpt"] = nc.dram_tensor("wpt", [C, C], F16, kind="ExternalInput").ap()
    io["ident"] = nc.dram_tensor("ident", [128, 128], F16, kind="ExternalInput").ap()
    io["mask"] = nc.dram_tensor("mask", [C, C], F32, kind="ExternalInput").ap()
    io["scale192"] = nc.dram_tensor("scale192", [C, 1], F32, kind="ExternalInput").ap()
    io["out"] = nc.dram_tensor("out", [C, HWTOT], F32, kind="ExternalOutput").ap()
    io["vdram"] = nc.dram_tensor("vdram", [C, HWTOT], F16).ap()
    io["kv16d"] = nc.dram_tensor("kv16d", [C, HWTOT], F16).ap()
    with tile.TileContext(nc) as tc:
        emit_kernel(tc, io)
    nc.compile()
    return nc


def prep_weights(qkv1_w, qkv2_w, proj_w, scale):
    w1 = np.asarray(qkv1_w).reshape(C2, C)
    w1t = np.ascontiguousarray(w1.T).astype(np.float16)
    w2 = np.asarray(qkv2_w).reshape(C2, 9)
    w2d = np.zeros((27, 128, 128), np.float16)
    for mc in range(3):
        for wi in range(9):
            np.fill_diagonal(w2d[mc * 9 + wi], w2[mc * 128:(mc + 1) * 128, wi])
    wp = np.asarray(proj_w).reshape(C, C)
    wpt = np.ascontiguousarray(wp.T).astype(np.float16)
    ident = np.eye(128, dtype=np.float16)
    mask = np.full((C, C), -1e30, np.float32)
    for h in range(HEADS):
        mask[h * CD:(h + 1) * CD, h * CD:(h + 1) * CD] = 0.0
    scale192 = np.repeat(np.asarray(scale).reshape(HEADS), CD).astype(
        np.float32).reshape(C, 1)
    return {"w1t": w1t, "w2d": w2d, "wpt": wpt, "ident": ident,
            "mask": mask, "scale192": scale192}


_CACHED = {}


def kernel(kv, q, qkv1_w, qkv2_w, proj_w, scale):
    kv = np.asarray(kv, np.float32)
    q = np.asarray(q, np.float32)
    b = kv.shape[0]
    assert b == 8 and kv.shape[1] == C
    wts = prep_weights(qkv1_w, qkv2_w, proj_w, scale)
    if "nc" not in _CACHED:
        nc = build_module()
        nc.m = get_hw_module(nc.m)
        _CACHED["nc"] = nc
    nc = _CACHED["nc"]
    in_maps = []
    for i in range(b):
        m = {"kv": np.ascontiguousarray(kv[i].reshape(C, HWTOT)),
             "q": np.ascontiguousarray(q[i].reshape(C, HWTOT))}
        m.update(wts)
        in_maps.append(m)
    res = run_bass_kernel_spmd(nc, in_maps, core_ids=list(range(8)))
    out = np.stack([res.results[i]["out"].reshape(C, H, W) for i in range(b)])
    return out.astype(np.float32)



# revision 10
# speedup vs baseline: 1.8645x; 1.8645x over previous
"""Trainium2 Bass kernel for nn_Cross_Attention (8-core data-parallel over batch).

v3 streaming design:
- SWDGE cast-DMAs load kv/q f32->f16 (no conversion pass).
- conv1 on PE; depthwise 3x3 split between PE (diagonal matmuls on flat
  wrap-around slices + fixups) and DVE (STT chains on a +1-shifted copy),
  per (chunk, 32-row slab).
- k/v produced slab-wise: k slabs DMA-xbar-transposed straight into kdT
  [pix, row, 192]; v slabs stored to DRAM f16 (reloaded in final pass).
- q streamed slab-wise: cast-load -> square-partials -> xbar transpose ->
  Gram accumulation; no full q/qT resident.
- L2 norms folded into softmax logits via S = outer(scale/|q|, 1/|k|)
  (tiny DRAM bounce to turn norm columns into rows).
- proj fused into attn@v: MT = (Wp @ A)^T precompute, one pass over v,
  out stored f16 (host casts to f32).
"""

import os
import sys
from contextlib import ExitStack

sys.path.insert(0, "/opt/trn_rl_repo")

import numpy as np

import concourse.bass as bass
import concourse.tile as tile
from concourse import bacc, mybir
from concourse.bass_utils import run_bass_kernel_spmd
from concourse.bass_interp import get_hw_module

F32 = mybir.dt.float32
F16 = mybir.dt.float16
MULT = mybir.AluOpType.mult
ADD = mybir.AluOpType.add
BYPASS = mybir.AluOpType.bypass
AX = mybir.AxisListType.X
AF = mybir.ActivationFunctionType

C = 192
C2 = 384
HEADS = 8
CD = C // HEADS
W = 128
H = int(os.environ.get("BASS_CA_H", "128"))
HWTOT = H * W
SLAB_R = 32
NS = H // SLAB_R
SLW = SLAB_R * W                 # 4096 pixels per slab
# kvf rows: 0 zero, 1 top-boundary, 2..33 interior, 34 bottom-boundary, 35 zero
KVF_R = SLAB_R + 4
PE_T = int(os.environ.get("BASS_CA_PET", "3"))   # 4-row tiles per slab on PE
assert 1 <= PE_T <= 8
EPS = 1e-12

TAPS = [(0, 0)] + [(dr, dc) for dr in (-1, 0, 1) for dc in (-1, 0, 1)
                   if not (dr == 0 and dc == 0)]


def emit_slab(tc, io, sb, mc, s):
    """conv1 + depthwise for chunk mc, slab s. Output lands in a rotating
    slab tile: mc0 -> kd_a rows, mc1 -> [kd_b ; vd_lo], mc2 -> vd_hi."""
    nc = tc.nc
    r0img = s * SLAB_R
    mcs = slice(mc * 128, (mc + 1) * 128)
    ssl = slice(s * SLW, (s + 1) * SLW)

    kvf = sb["kvfp"].tile([128, KVF_R * W], F16, tag="kvf", name="kvf")
    kvf3 = kvf[:].rearrange("p (r c) -> p r c", c=W)
    ds = sb["dsp"].tile([128, SLW], F16, tag="ds", name="ds")
    ds3 = ds[:].rearrange("p (r c) -> p r c", c=W)

    # ---- conv1 interior rows (kvf rows 2..33): 4 psum pairs of 8 rows ----
    for j in range(SLAB_R // 8):
        ps = sb["psc"].tile([128, 1024], F32, tag="psc", name="ps")
        for h in range(2):
            pix = (r0img + 8 * j + 4 * h) * W
            psl = ps[:, h * 512:(h + 1) * 512]
            nc.tensor.matmul(psl, sb["w1ta"][:, mcs], io["kv16a"][:, pix:pix + 512],
                             start=True, stop=False)
            nc.tensor.matmul(psl, sb["w1tb"][:, mcs], io["kv16b"][:, pix:pix + 512],
                             start=False, stop=True)
        nc.scalar.copy(kvf[:, (2 + 8 * j) * W:(10 + 8 * j) * W], ps[:])

    # ---- boundary rows 1 (img r0img-1) and 34 (img r0img+32); 0/35 zero ----
    nc.vector.memset(kvf3[:, 0, :], 0.0)
    nc.vector.memset(kvf3[:, KVF_R - 1, :], 0.0)
    needb = [(krow, img) for krow, img in
             ((1, r0img - 1), (SLAB_R + 2, r0img + SLAB_R)) if 0 <= img < H]
    for krow, img in ((1, r0img - 1), (SLAB_R + 2, r0img + SLAB_R)):
        if not (0 <= img < H):
            nc.vector.memset(kvf3[:, krow, :], 0.0)
    if needb:
        bps = sb["psc"].tile([128, 1024], F32, tag="psc", name="bps")
        for bi, (krow, img) in enumerate(needb):
            psl = bps[:, bi * 128:(bi + 1) * 128]
            pix = img * W
            nc.tensor.matmul(psl, sb["w1ta"][:, mcs], io["kv16a"][:, pix:pix + 128],
                             start=True, stop=False)
            nc.tensor.matmul(psl, sb["w1tb"][:, mcs], io["kv16b"][:, pix:pix + 128],
                             start=False, stop=True)
            nc.scalar.copy(kvf3[:, krow, :], psl)

    # ---- PE depthwise tiles (flat wrap-around slices, fixups after) ----
    for tp in range(0, PE_T, 2):
        tn = min(2, PE_T - tp)
        ps = sb["psd"].tile([128, 1024], F32, tag="psd", name="psd")
        for t in range(tp, tp + tn):
            psl = ps[:, (t - tp) * 512:(t - tp + 1) * 512]
            rk = 2 + 4 * t
            for ti, (dr, dc) in enumerate(TAPS):
                wi = (dr + 1) * 3 + (dc + 1)
                lw = sb["w2sb"][:, mc * 9 + wi, :]
                base = (rk + dr) * W + dc
                nc.tensor.matmul(psl, lw, kvf[:, base:base + 512],
                                 start=(ti == 0), stop=(ti == 8))
        nc.scalar.copy(ds[:, tp * 512:(tp + tn) * 512], ps[:, 0:tn * 512])
    npe = 4 * PE_T
    for (dr, dc) in TAPS:
        if dc == 0:
            continue
        wi = (dr + 1) * 3 + (dc + 1)
        wnap = sb["w2vn"][:, mc * 9 + wi: mc * 9 + wi + 1]
        if dc == 1:
            badcol, src3 = 127, kvf3[:, 3 + dr: 3 + dr + npe, 0:1]
        else:
            badcol, src3 = 0, kvf3[:, 1 + dr: 1 + dr + npe, 127:128]
        nc.vector.scalar_tensor_tensor(
            out=ds3[:, 0:npe, badcol:badcol + 1], in0=src3, scalar=wnap,
            in1=ds3[:, 0:npe, badcol:badcol + 1], op0=MULT, op1=ADD)

    # ---- DVE depthwise rows 4*PE_T..31 ----
    nr = SLAB_R - 4 * PE_T
    if nr > 0:
        fd = nr * W
        kvfs = sb["kvsp"].tile([128, KVF_R * W + 4], F16, tag="kvfs", name="kvfs")
        nc.sync.dma_start(kvfs[:, 2:2 + KVF_R * W - 1], kvf[:, 1:KVF_R * W])
        nc.vector.memset(kvfs[:, 0:2], 0.0)
        nc.vector.memset(kvfs[:, 2 + KVF_R * W - 1:], 0.0)
        rk = 2 + 4 * PE_T
        acc = [sb["accp"].tile([128, fd], F16, tag=f"acc{i}", name=f"acc{i}")
               for i in range(2)]
        dvout = ds[:, npe * W:SLW]
        for ti, (dr, dc) in enumerate(TAPS):
            wi = (dr + 1) * 3 + (dc + 1)
            wap = sb["w2v"][:, mc * 9 + wi: mc * 9 + wi + 1]
            if dc == 0:
                src = kvf[:, (rk + dr) * W:(rk + dr) * W + fd]
            elif dc == 1:
                src = kvfs[:, 2 + (rk + dr) * W:2 + (rk + dr) * W + fd]
            else:
                src = kvfs[:, (rk + dr) * W:(rk + dr) * W + fd]
            out = dvout if ti == 8 else acc[(ti + 1) % 2]
            if ti == 0:
                nc.vector.tensor_scalar_mul(out[:], src, wap)
            else:
                nc.vector.scalar_tensor_tensor(out=out[:], in0=src, scalar=wap,
                                               in1=acc[ti % 2][:], op0=MULT, op1=ADD)
        for (dr, dc) in TAPS:
            if dc == 0:
                continue
            wi = (dr + 1) * 3 + (dc + 1)
            wnap = sb["w2vn"][:, mc * 9 + wi: mc * 9 + wi + 1]
            if dc == 1:
                badcol, src3 = 127, kvf3[:, rk + dr + 1: rk + dr + 1 + nr, 0:1]
            else:
                badcol, src3 = 0, kvf3[:, rk + dr - 1: rk + dr - 1 + nr, 127:128]
            nc.vector.scalar_tensor_tensor(
                out=ds3[:, npe:SLAB_R, badcol:badcol + 1], in0=src3, scalar=wnap,
                in1=ds3[:, npe:SLAB_R, badcol:badcol + 1], op0=MULT, op1=ADD)

    # ---- consume the slab: transposes to kdT / stores to vdram / norms ----
    kdT, nqk = sb["kdT"], sb["nqk"]
    if mc == 0:
        junk = sb["kvfp"].tile([128, KVF_R * W], F16, tag="kvf", name="junka")
        nc.scalar.activation(junk[:, 0:SLW], ds[:], AF.Square,
                             accum_out=nqk["ka"][:, s:s + 1])
        nc.sync.dma_start_transpose(kdT[:, s * SLAB_R:(s + 1) * SLAB_R, 0:128],
                                    ds[:])
    elif mc == 1:
        junk = sb["kvfp"].tile([128, KVF_R * W], F16, tag="kvf", name="junkb")
        nc.vector.scalar_tensor_tensor(
            out=junk[0:64, 0:SLW], in0=ds[0:64, :], scalar=1.0, in1=ds[0:64, :],
            op0=BYPASS, op1=MULT, accum_out=nqk["kb"][:, s:s + 1])
        nc.sync.dma_start_transpose(kdT[:, s * SLAB_R:(s + 1) * SLAB_R, 128:192],
                                    ds[0:64, :])
        nc.gpsimd.dma_start(io["vdram"][0:64, ssl], ds[64:128, :])
    else:
        nc.gpsimd.dma_start(io["vdram"][64:C, ssl], ds[:])


def emit_kernel(tc, io):
    nc = tc.nc
    st = ExitStack()
    wp = st.enter_context(tc.tile_pool(name="weights", bufs=1))
    sb = {}

    for nm, src, shape, dt in (
            ("w1ta", io["w1t"][0:128, :], [128, C2], F16),
            ("w1tb", io["w1t"][128:C, :], [64, C2], F16),
            ("w2v", io["w2v"][:], [128, 27], F32),
            ("w2vn", io["w2vn"][:], [128, 27], F32),
            ("wpta", io["wpt"][0:128, :], [128, C], F16),
            ("wptb", io["wpt"][128:C, :], [64, C], F16),
            ("maska", io["mask"][0:128, :], [128, C], F32),
            ("maskb", io["mask"][128:C, :], [64, C], F32),
            ("sca", io["scale192"][0:128, :], [128, 1], F32),
            ("scb", io["scale192"][128:C, :], [64, 1], F32)):
        sb[nm] = wp.tile(shape, dt, name=nm)
        nc.sync.dma_start(sb[nm][:], src)
    sb["w2sb"] = wp.tile([128, 27, 128], F16, name="w2sb")
    nc.sync.dma_start(sb["w2sb"][:], io["w2d"].rearrange("t p c -> p t c"))

    sb["kdT"] = wp.tile([128, H, 192], F16, name="kdT")
    sb["nqk"] = {"ka": wp.tile([128, NS], F32, name="nq_ka"),
                 "kb": wp.tile([64, NS], F32, name="nq_kb"),
                 "qa": wp.tile([128, NS], F32, name="nq_qa"),
                 "qb": wp.tile([64, NS], F32, name="nq_qb")}
    mt1 = wp.tile([64, C], F16, name="mt1")
    mt2 = wp.tile([128, C], F16, name="mt2")

    # ================= phase A: conv1 + depthwise, k/v production =========
    with tc.tile_pool(name="kvp", bufs=1) as kvp, \
         tc.tile_pool(name="kvfp", bufs=2) as kvfp, \
         tc.tile_pool(name="kvsp", bufs=2) as kvsp, \
         tc.tile_pool(name="accp", bufs=1) as accp, \
         tc.tile_pool(name="dsp", bufs=3) as dsp, \
         tc.tile_pool(name="psc", bufs=2, space="PSUM") as psc, \
         tc.tile_pool(name="psd", bufs=2, space="PSUM") as psd:
        sb.update({"kvfp": kvfp, "kvsp": kvsp, "accp": accp, "dsp": dsp,
                   "psc": psc, "psd": psd})
        io["kv16a"] = kvp.tile([128, HWTOT], F16, name="kv16a")
        io["kv16b"] = kvp.tile([64, HWTOT], F16, name="kv16b")
        for s in range(NS):
            ssl = slice(s * SLW, (s + 1) * SLW)
            nc.gpsimd.dma_start(io["kv16a"][:, ssl], io["kv"][0:128, ssl])
            nc.gpsimd.dma_start(io["kv16b"][:, ssl], io["kv"][128:C, ssl])
        for mc in range(3):
            for s in range(NS):
                emit_slab(tc, io, sb, mc, s)

    # ================= phase B: q stream + Gram + softmax + MT ============
    nqk = sb["nqk"]
    with tc.tile_pool(name="qsp", bufs=2) as qsp, \
         tc.tile_pool(name="qtp", bufs=2) as qtp, \
         tc.tile_pool(name="smx", bufs=1) as smx, \
         tc.tile_pool(name="psg", bufs=1, space="PSUM") as psg:
        G0 = psg.tile([128, C], F32, tag="G0", name="G0")
        G1 = psg.tile([64, C], F32, tag="G1", name="G1")
        for s in range(NS):
            ssl = slice(s * SLW, (s + 1) * SLW)
            qsa = qsp.tile([128, SLW], F16, tag="qsa", name="qsa")
            nc.gpsimd.dma_start(qsa[:], io["q"][0:128, ssl])
            qsb = qsp.tile([64, SLW], F16, tag="qsb", name="qsb")
            nc.gpsimd.dma_start(qsb[:], io["q"][128:C, ssl])
            jq = qtp.tile([128, SLW], F16, tag="jq", name="jq")
            nc.scalar.activation(jq[:], qsa[:], AF.Square,
                                 accum_out=nqk["qa"][:, s:s + 1])
            nc.vector.scalar_tensor_tensor(
                out=jq[0:64, :], in0=qsb[:], scalar=1.0, in1=qsb[:],
                op0=BYPASS, op1=MULT, accum_out=nqk["qb"][:, s:s + 1])
            qta = qtp.tile([128, SLAB_R, 128], F16, tag="qta", name="qta")
            nc.sync.dma_start_transpose(qta[:], qsa[:])
            qtb = qtp.tile([128, SLAB_R, 64], F16, tag="qtb", name="qtb")
            nc.sync.dma_start_transpose(qtb[:], qsb[:])
            for t in range(SLAB_R):
                tg = s * SLAB_R + t
                nc.tensor.matmul(G0[:], qta[:, t, :], sb["kdT"][:, tg, :],
                                 start=(tg == 0), stop=(tg == H - 1))
                nc.tensor.matmul(G1[:], qtb[:, t, :], sb["kdT"][:, tg, :],
                                 start=(tg == 0), stop=(tg == H - 1))

        # ---- norms -> S = outer(scale/|q|, 1/|k|) via tiny DRAM bounce ----
        sqa = smx.tile([128, 1], F32, name="sqa")
        sqb = smx.tile([64, 1], F32, name="sqb")
        ska = smx.tile([128, 1], F32, name="ska")
        skb = smx.tile([64, 1], F32, name="skb")
        for dst, part, scl in ((sqa, "qa", sb["sca"]), (sqb, "qb", sb["scb"]),
                               (ska, "ka", None), (skb, "kb", None)):
            nc.vector.reduce_sum(dst[:], nqk[part][:], axis=AX)
            nc.scalar.sqrt(dst[:], dst[:])
            nc.vector.tensor_scalar_max(dst[:], dst[:], EPS)
            nc.vector.reciprocal(dst[:], dst[:])
            if scl is not None:
                nc.vector.tensor_tensor(out=dst[:], in0=dst[:], in1=scl[:],
                                        op=MULT)
        nc.sync.dma_start(io["nrm"][0:1, 0:128], sqa[:])
        nc.sync.dma_start(io["nrm"][0:1, 128:C], sqb[:])
        nc.sync.dma_start(io["nrm"][1:2, 0:128], ska[:])
        nc.sync.dma_start(io["nrm"][1:2, 128:C], skb[:])
        sqra = smx.tile([1, 128], F32, name="sqra")
        nc.sync.dma_start(sqra[:], io["nrm"][0:1, 0:128])
        sqrb = smx.tile([1, 64], F32, name="sqrb")
        nc.sync.dma_start(sqrb[:], io["nrm"][0:1, 128:C])
        skr = smx.tile([1, C], F32, name="skr")
        nc.sync.dma_start(skr[:], io["nrm"][1:2, :])

        at = {}
        with tc.tile_pool(name="pss", bufs=1, space="PSUM") as pss:
            S0 = pss.tile([128, C], F32, tag="S0", name="S0")
            S1 = pss.tile([64, C], F32, tag="S1", name="S1")
            nc.tensor.matmul(S0[:], sqra[:], skr[:], start=True, stop=True)
            nc.tensor.matmul(S1[:], sqrb[:], skr[:], start=True, stop=True)
            for nm, G, S, mk, rows in (("a", G0, S0, sb["maska"], 128),
                                       ("b", G1, S1, sb["maskb"], 64)):
                ssb = smx.tile([rows, C], F32, name=f"ssb{nm}")
                nc.scalar.copy(ssb[:], S[:])
                lg = smx.tile([rows, C], F32, name=f"lg{nm}")
                nc.vector.tensor_tensor(out=lg[:], in0=G[:], in1=ssb[:], op=MULT)
                nc.vector.tensor_tensor(out=lg[:], in0=lg[:], in1=mk[:], op=ADD)
                mx = smx.tile([rows, 1], F32, name=f"mx{nm}")
                nc.vector.reduce_max(mx[:], lg[:], axis=AX)
                nc.vector.tensor_scalar_mul(mx[:], mx[:], -1.0)
                ssum = smx.tile([rows, 1], F32, name=f"ss{nm}")
                nc.scalar.activation(lg[:], lg[:], AF.Exp, bias=mx[:],
                                     accum_out=ssum[:])
                nc.vector.reciprocal(ssum[:], ssum[:])
                a16 = smx.tile([rows, C], F16, name=f"a16{nm}")
                nc.vector.tensor_scalar_mul(a16[:], lg[:], ssum[:])
                at[nm] = a16

        with tc.tile_pool(name="psm", bufs=1, space="PSUM") as psm:
            MT0 = psm.tile([128, C], F32, tag="MT0", name="MT0")
            MT1 = psm.tile([64, C], F32, tag="MT1", name="MT1")
            nc.tensor.matmul(MT0[:], at["a"][:, 0:128], sb["wpta"][:],
                             start=True, stop=False)
            nc.tensor.matmul(MT0[:], at["b"][:, 0:128], sb["wptb"][:],
                             start=False, stop=True)
            nc.tensor.matmul(MT1[:], at["a"][:, 128:C], sb["wpta"][:],
                             start=True, stop=False)
            nc.tensor.matmul(MT1[:], at["b"][:, 128:C], sb["wptb"][:],
                             start=False, stop=True)
            nc.vector.tensor_copy(mt1[:], MT0[0:64, :])
            nc.vector.tensor_copy(mt2[0:64, :], MT0[64:128, :])
            nc.vector.tensor_copy(mt2[64:128, :], MT1[:])

    # ================= phase C: O = (Wp @ A) @ vd =========================
    NG = HWTOT // 2048
    with tc.tile_pool(name="vst", bufs=3) as vst, \
         tc.tile_pool(name="ost", bufs=3) as ost, \
         tc.tile_pool(name="pso", bufs=2, space="PSUM") as pso:
        for g in range(NG):
            gsl = slice(g * 2048, (g + 1) * 2048)
            vA = vst.tile([64, 2048], F16, tag="vA", name="vA")
            nc.sync.dma_start(vA[:], io["vdram"][0:64, gsl])
            vB = vst.tile([128, 2048], F16, tag="vB", name="vB")
            nc.sync.dma_start(vB[:], io["vdram"][64:C, gsl])
            for hp in range(2):
                O0 = pso.tile([128, 1024], F32, tag="O0", name="O0")
                O1 = pso.tile([64, 1024], F32, tag="O1", name="O1")
                for t in range(2):
                    vsl = slice(hp * 1024 + t * 512, hp * 1024 + (t + 1) * 512)
                    osl = slice(t * 512, (t + 1) * 512)
                    nc.tensor.matmul(O0[:, osl], mt1[:, 0:128], vA[:, vsl],
                                     start=True, stop=False)
                    nc.tensor.matmul(O0[:, osl], mt2[:, 0:128], vB[:, vsl],
                                     start=False, stop=True)
                    nc.tensor.matmul(O1[:, osl], mt1[:, 128:C], vA[:, vsl],
                                     start=True, stop=False)
                    nc.tensor.matmul(O1[:, osl], mt2[:, 128:C], vB[:, vsl],
                                     start=False, stop=True)
                oa = ost.tile([128, 1024], F16, tag="oa", name="oa")
                ob = ost.tile([64, 1024], F16, tag="ob", name="ob")
                nc.scalar.copy(oa[:], O0[:])
                nc.vector.tensor_copy(ob[:], O1[:])
                psl = slice(g * 2048 + hp * 1024, g * 2048 + (hp + 1) * 1024)
                nc.gpsimd.dma_start(io["out"][0:128, psl], oa[:])
                nc.gpsimd.dma_start(io["out"][128:C, psl], ob[:])
    st.close()


def build_module():
    nc = bacc.Bacc("TRN2")
    io = {}
    io["kv"] = nc.dram_tensor("kv", [C, HWTOT], F32, kind="ExternalInput").ap()
    io["q"] = nc.dram_tensor("q", [C, HWTOT], F32, kind="ExternalInput").ap()
    io["w1t"] = nc.dram_tensor("w1t", [C, C2], F16, kind="ExternalInput").ap()
    io["w2d"] = nc.dram_tensor("w2d", [27, 128, 128], F16, kind="ExternalInput").ap()
    io["w2v"] = nc.dram_tensor("w2v", [128, 27], F32, kind="ExternalInput").ap()
    io["w2vn"] = nc.dram_tensor("w2vn", [128, 27], F32, kind="ExternalInput").ap()
    io["wpt"] = nc.dram_tensor("wpt", [C, C], F16, kind="ExternalInput").ap()
    io["mask"] = nc.dram_tensor("mask", [C, C], F32, kind="ExternalInput").ap()
    io["scale192"] = nc.dram_tensor("scale192", [C, 1], F32, kind="ExternalInput").ap()
    io["out"] = nc.dram_tensor("out", [C, HWTOT], F16, kind="ExternalOutput").ap()
    io["vdram"] = nc.dram_tensor("vdram", [C, HWTOT], F16).ap()
    io["nrm"] = nc.dram_tensor("nrm", [2, C], F32).ap()
    with tile.TileContext(nc) as tc:
        emit_kernel(tc, io)
    nc.compile()
    return nc


def prep_weights(qkv1_w, qkv2_w, proj_w, scale):
    w1 = np.asarray(qkv1_w).reshape(C2, C)
    w1t = np.ascontiguousarray(w1.T).astype(np.float16)
    w2 = np.asarray(qkv2_w).reshape(C2, 9)
    w2d = np.zeros((27, 128, 128), np.float16)
    for mc in range(3):
        for wi in range(9):
            np.fill_diagonal(w2d[mc * 9 + wi], w2[mc * 128:(mc + 1) * 128, wi])
    w2v = np.zeros((128, 27), np.float32)
    for mc in range(3):
        w2v[:, mc * 9:(mc + 1) * 9] = w2[mc * 128:(mc + 1) * 128, :]
    wpt = np.ascontiguousarray(np.asarray(proj_w).reshape(C, C).T).astype(np.float16)
    mask = np.full((C, C), -1e30, np.float32)
    for h in range(HEADS):
        mask[h * CD:(h + 1) * CD, h * CD:(h + 1) * CD] = 0.0
    scale192 = np.repeat(np.asarray(scale).reshape(HEADS), CD).astype(
        np.float32).reshape(C, 1)
    return {"w1t": w1t, "w2d": w2d, "w2v": w2v, "w2vn": -w2v, "wpt": wpt,
            "mask": mask, "scale192": scale192}


_CACHED = {}


def kernel(kv, q, qkv1_w, qkv2_w, proj_w, scale):
    kv = np.asarray(kv, np.float32)
    q = np.asarray(q, np.float32)
    b = kv.shape[0]
    assert b == 8 and kv.shape[1] == C
    wts = prep_weights(qkv1_w, qkv2_w, proj_w, scale)
    if "nc" not in _CACHED:
        nc = build_module()
        nc.m = get_hw_module(nc.m)
        _CACHED["nc"] = nc
    nc = _CACHED["nc"]
    in_maps = []
    for i in range(b):
        m = {"kv": np.ascontiguousarray(kv[i].reshape(C, HWTOT)),
             "q": np.ascontiguousarray(q[i].reshape(C, HWTOT))}
        m.update(wts)
        in_maps.append(m)
    res = run_bass_kernel_spmd(nc, in_maps, core_ids=list(range(8)))
    out = np.stack([res.results[i]["out"].reshape(C, H, W) for i in range(b)])
    return out.astype(np.float32)


# revision 14
# speedup vs baseline: 3.1706x; 1.7006x over previous
"""Trainium2 Bass kernel for nn_Cross_Attention (8-core data-parallel over batch).

v3 streaming design:
- SWDGE cast-DMAs load kv/q f32->f16 (no conversion pass).
- conv1 on PE; depthwise 3x3 split between PE (diagonal matmuls on flat
  wrap-around slices + fixups) and DVE (STT chains on a +1-shifted copy),
  per (chunk, 32-row slab).
- k/v produced slab-wise: k slabs DMA-xbar-transposed straight into kdT
  [pix, row, 192]; v slabs stored to DRAM f16 (reloaded in final pass).
- q streamed slab-wise: cast-load -> square-partials -> xbar transpose ->
  Gram accumulation; no full q/qT resident.
- L2 norms folded into softmax logits via S = outer(scale/|q|, 1/|k|)
  (tiny DRAM bounce to turn norm columns into rows).
- proj fused into attn@v: MT = (Wp @ A)^T precompute, one pass over v,
  out stored f16 (host casts to f32).
"""

import os
import sys
from contextlib import ExitStack

sys.path.insert(0, "/opt/trn_rl_repo")

import numpy as np

import concourse.bass as bass
import concourse.tile as tile
from concourse import bacc, mybir
from concourse.bass_utils import run_bass_kernel_spmd
from concourse.bass_interp import get_hw_module

F32 = mybir.dt.float32
F16 = mybir.dt.float16
MULT = mybir.AluOpType.mult
ADD = mybir.AluOpType.add
BYPASS = mybir.AluOpType.bypass
AX = mybir.AxisListType.X
AF = mybir.ActivationFunctionType

C = 192
C2 = 384
HEADS = 8
CD = C // HEADS
W = 128
H = int(os.environ.get("BASS_CA_H", "128"))
HWTOT = H * W
SLAB_R = 32
NS = H // SLAB_R
SLW = SLAB_R * W                 # 4096 pixels per slab
# kvf rows: 0 zero, 1 top-boundary, 2..33 interior, 34 bottom-boundary, 35 zero
KVF_R = SLAB_R + 4
PE_T = int(os.environ.get("BASS_CA_PET", "5"))   # 4-row tiles per slab on PE
assert 1 <= PE_T <= 8
EPS = 1e-12

TAPS = [(0, 0)] + [(dr, dc) for dr in (-1, 0, 1) for dc in (-1, 0, 1)
                   if not (dr == 0 and dc == 0)]


def emit_slab(tc, io, sb, mc, s):
    """conv1 + depthwise for chunk mc, slab s. Output lands in a rotating
    slab tile: mc0 -> kd_a rows, mc1 -> [kd_b ; vd_lo], mc2 -> vd_hi."""
    nc = tc.nc
    r0img = s * SLAB_R
    mcs = slice(mc * 128, (mc + 1) * 128)
    ssl = slice(s * SLW, (s + 1) * SLW)

    kvf = sb["kvfp"].tile([128, KVF_R * W], F16, tag="kvf", name="kvf")
    kvf3 = kvf[:].rearrange("p (r c) -> p r c", c=W)
    ds = sb["dsp"].tile([128, SLW], F16, tag="ds", name="ds")
    ds3 = ds[:].rearrange("p (r c) -> p r c", c=W)

    # ---- conv1 interior rows (kvf rows 2..33): 4 psum pairs of 8 rows ----
    for j in range(SLAB_R // 8):
        ps = sb["psc"].tile([128, 1024], F32, tag="psc", name="ps")
        for h in range(2):
            pix = (r0img + 8 * j + 4 * h) * W
            psl = ps[:, h * 512:(h + 1) * 512]
            nc.tensor.matmul(psl, sb["w1ta"][:, mcs], io["kv16a"][:, pix:pix + 512],
                             start=True, stop=False)
            nc.tensor.matmul(psl, sb["w1tb"][:, mcs], io["kv16b"][:, pix:pix + 512],
                             start=False, stop=True)
        nc.scalar.copy(kvf[:, (2 + 8 * j) * W:(10 + 8 * j) * W], ps[:])

    # ---- boundary rows 1 (img r0img-1) and 34 (img r0img+32); 0/35 zero ----
    nc.vector.memset(kvf3[:, 0, :], 0.0)
    nc.vector.memset(kvf3[:, KVF_R - 1, :], 0.0)
    needb = [(krow, img) for krow, img in
             ((1, r0img - 1), (SLAB_R + 2, r0img + SLAB_R)) if 0 <= img < H]
    for krow, img in ((1, r0img - 1), (SLAB_R + 2, r0img + SLAB_R)):
        if not (0 <= img < H):
            nc.vector.memset(kvf3[:, krow, :], 0.0)
    if needb:
        bps = sb["psc"].tile([128, 1024], F32, tag="psc", name="bps")
        for bi, (krow, img) in enumerate(needb):
            psl = bps[:, bi * 128:(bi + 1) * 128]
            pix = img * W
            nc.tensor.matmul(psl, sb["w1ta"][:, mcs], io["kv16a"][:, pix:pix + 128],
                             start=True, stop=False)
            nc.tensor.matmul(psl, sb["w1tb"][:, mcs], io["kv16b"][:, pix:pix + 128],
                             start=False, stop=True)
            nc.scalar.copy(kvf3[:, krow, :], psl)

    # ---- PE depthwise tiles (flat wrap-around slices, fixups after) ----
    for tp in range(0, PE_T, 2):
        tn = min(2, PE_T - tp)
        ps = sb["psd"].tile([128, 1024], F32, tag="psd", name="psd")
        for t in range(tp, tp + tn):
            psl = ps[:, (t - tp) * 512:(t - tp + 1) * 512]
            rk = 2 + 4 * t
            for ti, (dr, dc) in enumerate(TAPS):
                wi = (dr + 1) * 3 + (dc + 1)
                lw = sb["w2sb"][:, mc * 9 + wi, :]
                base = (rk + dr) * W + dc
                nc.tensor.matmul(psl, lw, kvf[:, base:base + 512],
                                 start=(ti == 0), stop=(ti == 8))
        nc.scalar.copy(ds[:, tp * 512:(tp + tn) * 512], ps[:, 0:tn * 512])
    npe = 4 * PE_T
    for (dr, dc) in TAPS:
        if dc == 0:
            continue
        wi = (dr + 1) * 3 + (dc + 1)
        wnap = sb["w2vn"][:, mc * 9 + wi: mc * 9 + wi + 1]
        if dc == 1:
            badcol, src3 = 127, kvf3[:, 3 + dr: 3 + dr + npe, 0:1]
        else:
            badcol, src3 = 0, kvf3[:, 1 + dr: 1 + dr + npe, 127:128]
        nc.vector.scalar_tensor_tensor(
            out=ds3[:, 0:npe, badcol:badcol + 1], in0=src3, scalar=wnap,
            in1=ds3[:, 0:npe, badcol:badcol + 1], op0=MULT, op1=ADD)

    # ---- DVE depthwise rows 4*PE_T..31 ----
    nr = SLAB_R - 4 * PE_T
    if nr > 0:
        fd = nr * W
        rk = 2 + 4 * PE_T
        acc = [sb["accp"].tile([128, fd], F16, tag=f"acc{i}", name=f"acc{i}")
               for i in range(2)]
        tmp = sb["accp"].tile([128, fd], F16, tag="tmp", name="tmp")
        dvout = ds[:, npe * W:SLW]
        for ti, (dr, dc) in enumerate(TAPS):
            wi = (dr + 1) * 3 + (dc + 1)
            wap = sb["w2v"][:, mc * 9 + wi: mc * 9 + wi + 1]
            base = (rk + dr) * W + dc
            src = kvf[:, base:base + fd]
            if ti == 0:
                nc.vector.tensor_scalar_mul(acc[1][:], src, wap)
            else:
                out = dvout if ti == 8 else acc[(ti + 1) % 2]
                nc.vector.tensor_scalar_mul(tmp[:], src, wap)
                nc.vector.tensor_tensor(out=out[:], in0=tmp[:],
                                        in1=acc[ti % 2][:], op=ADD)
        for (dr, dc) in TAPS:
            if dc == 0:
                continue
            wi = (dr + 1) * 3 + (dc + 1)
            wnap = sb["w2vn"][:, mc * 9 + wi: mc * 9 + wi + 1]
            if dc == 1:
                badcol, src3 = 127, kvf3[:, rk + dr + 1: rk + dr + 1 + nr, 0:1]
            else:
                badcol, src3 = 0, kvf3[:, rk + dr - 1: rk + dr - 1 + nr, 127:128]
            nc.vector.scalar_tensor_tensor(
                out=ds3[:, npe:SLAB_R, badcol:badcol + 1], in0=src3, scalar=wnap,
                in1=ds3[:, npe:SLAB_R, badcol:badcol + 1], op0=MULT, op1=ADD)

    # ---- consume the slab: transposes to kdT / stores to vdram / norms ----
    kdT, nqk = sb["kdT"], sb["nqk"]
    if mc == 0:
        junk = sb["kvfp"].tile([128, SLW], F16, tag="junk", bufs=1, name="junka")
        nc.scalar.activation(junk[:], ds[:], AF.Square,
                             accum_out=nqk["ka"][:, s:s + 1])
        nc.sync.dma_start_transpose(kdT[:, s * SLAB_R:(s + 1) * SLAB_R, 0:128],
                                    ds[:])
    elif mc == 1:
        junk = sb["kvfp"].tile([128, SLW], F16, tag="junk", bufs=1, name="junkb")
        nc.vector.scalar_tensor_tensor(
            out=junk[0:64, :], in0=ds[0:64, :], scalar=1.0, in1=ds[0:64, :],
            op0=BYPASS, op1=MULT, accum_out=nqk["kb"][:, s:s + 1])
        nc.sync.dma_start_transpose(kdT[:, s * SLAB_R:(s + 1) * SLAB_R, 128:192],
                                    ds[0:64, :])
        nc.scalar.dma_start(io["vdram"][0:64, ssl], ds[64:128, :])
    else:
        nc.scalar.dma_start(io["vdram"][64:C, ssl], ds[:])


def emit_kernel(tc, io):
    nc = tc.nc
    st = ExitStack()
    wp = st.enter_context(tc.tile_pool(name="weights", bufs=1))
    sb = {}

    for nm, src, shape, dt in (
            ("w1ta", io["w1t"][0:128, :], [128, C2], F16),
            ("w1tb", io["w1t"][128:C, :], [64, C2], F16),
            ("w2v", io["w2v"][:], [128, 27], F32),
            ("w2vn", io["w2vn"][:], [128, 27], F32),
            ("wpta", io["wpt"][0:128, :], [128, C], F16),
            ("wptb", io["wpt"][128:C, :], [64, C], F16),
            ("maska", io["mask"][0:128, :], [128, C], F32),
            ("maskb", io["mask"][128:C, :], [64, C], F32),
            ("sca", io["scale192"][0:128, :], [128, 1], F32),
            ("scb", io["scale192"][128:C, :], [64, 1], F32)):
        sb[nm] = wp.tile(shape, dt, name=nm)
        nc.sync.dma_start(sb[nm][:], src)
    sb["w2sb"] = wp.tile([128, 27, 128], F16, name="w2sb")
    nc.sync.dma_start(sb["w2sb"][:], io["w2d"].rearrange("t p c -> p t c"))

    sb["kdT"] = wp.tile([128, H, 192], F16, name="kdT")
    sb["nqk"] = {"ka": wp.tile([128, NS], F32, name="nq_ka"),
                 "kb": wp.tile([64, NS], F32, name="nq_kb"),
                 "qa": wp.tile([128, NS], F32, name="nq_qa"),
                 "qb": wp.tile([64, NS], F32, name="nq_qb")}
    mt1 = wp.tile([64, C], F16, name="mt1")
    mt2 = wp.tile([128, C], F16, name="mt2")

    # ================= phase A: conv1 + depthwise, k/v production =========
    with tc.tile_pool(name="kvp", bufs=1) as kvp, \
         tc.tile_pool(name="kvfp", bufs=2) as kvfp, \
         tc.tile_pool(name="accp", bufs=1) as accp, \
         tc.tile_pool(name="dsp", bufs=4) as dsp, \
         tc.tile_pool(name="psc", bufs=2, space="PSUM") as psc, \
         tc.tile_pool(name="psd", bufs=2, space="PSUM") as psd:
        sb.update({"kvfp": kvfp, "accp": accp, "dsp": dsp,
                   "psc": psc, "psd": psd})
        io["kv16a"] = kvp.tile([128, HWTOT], F16, name="kv16a")
        io["kv16b"] = kvp.tile([64, HWTOT], F16, name="kv16b")
        for s in range(NS):
            ssl = slice(s * SLW, (s + 1) * SLW)
            nc.sync.dma_start(io["kv16a"][:, ssl], io["kv"][0:128, ssl])
            nc.sync.dma_start(io["kv16b"][:, ssl], io["kv"][128:C, ssl])
        for mc in range(3):
            for s in range(NS):
                emit_slab(tc, io, sb, mc, s)

    # ================= phase B: q stream + Gram + softmax + MT ============
    nqk = sb["nqk"]
    with tc.tile_pool(name="qsp", bufs=2) as qsp, \
         tc.tile_pool(name="qtp", bufs=2) as qtp, \
         tc.tile_pool(name="smx", bufs=1) as smx, \
         tc.tile_pool(name="psg", bufs=1, space="PSUM") as psg:
        G0 = psg.tile([128, C], F32, tag="G0", name="G0")
        G1 = psg.tile([64, C], F32, tag="G1", name="G1")
        for s in range(NS):
            ssl = slice(s * SLW, (s + 1) * SLW)
            qsa = qsp.tile([128, SLW], F16, tag="qsa", name="qsa")
            nc.scalar.dma_start(qsa[:], io["q"][0:128, ssl])
            qsb = qsp.tile([64, SLW], F16, tag="qsb", name="qsb")
            nc.scalar.dma_start(qsb[:], io["q"][128:C, ssl])
            jq = qtp.tile([128, SLW], F16, tag="jq", name="jq")
            nc.scalar.activation(jq[:], qsa[:], AF.Square,
                                 accum_out=nqk["qa"][:, s:s + 1])
            nc.vector.scalar_tensor_tensor(
                out=jq[0:64, :], in0=qsb[:], scalar=1.0, in1=qsb[:],
                op0=BYPASS, op1=MULT, accum_out=nqk["qb"][:, s:s + 1])
            qta = qtp.tile([128, SLAB_R, 128], F16, tag="qta", name="qta")
            nc.sync.dma_start_transpose(qta[:], qsa[:])
            qtb = qtp.tile([128, SLAB_R, 64], F16, tag="qtb", name="qtb")
            nc.sync.dma_start_transpose(qtb[:], qsb[:])
            for t in range(SLAB_R):
                tg = s * SLAB_R + t
                nc.tensor.matmul(G0[:], qta[:, t, :], sb["kdT"][:, tg, :],
                                 start=(tg == 0), stop=(tg == H - 1))
                nc.tensor.matmul(G1[:], qtb[:, t, :], sb["kdT"][:, tg, :],
                                 start=(tg == 0), stop=(tg == H - 1))

        # ---- norms -> S = outer(scale/|q|, 1/|k|) via tiny DRAM bounce ----
        sqa = smx.tile([128, 1], F32, name="sqa")
        sqb = smx.tile([64, 1], F32, name="sqb")
        ska = smx.tile([128, 1], F32, name="ska")
        skb = smx.tile([64, 1], F32, name="skb")
        for dst, part, scl in ((sqa, "qa", sb["sca"]), (sqb, "qb", sb["scb"]),
                               (ska, "ka", None), (skb, "kb", None)):
            nc.vector.reduce_sum(dst[:], nqk[part][:], axis=AX)
            nc.scalar.sqrt(dst[:], dst[:])
            nc.vector.tensor_scalar_max(dst[:], dst[:], EPS)
            nc.vector.reciprocal(dst[:], dst[:])
            if scl is not None:
                nc.vector.tensor_tensor(out=dst[:], in0=dst[:], in1=scl[:],
                                        op=MULT)
        nc.sync.dma_start(io["nrm"][0:1, 0:128], sqa[:])
        nc.sync.dma_start(io["nrm"][0:1, 128:C], sqb[:])
        nc.sync.dma_start(io["nrm"][1:2, 0:128], ska[:])
        nc.sync.dma_start(io["nrm"][1:2, 128:C], skb[:])
        sqra = smx.tile([1, 128], F32, name="sqra")
        nc.sync.dma_start(sqra[:], io["nrm"][0:1, 0:128])
        sqrb = smx.tile([1, 64], F32, name="sqrb")
        nc.sync.dma_start(sqrb[:], io["nrm"][0:1, 128:C])
        skr = smx.tile([1, C], F32, name="skr")
        nc.sync.dma_start(skr[:], io["nrm"][1:2, :])

        at = {}
        with tc.tile_pool(name="pss", bufs=1, space="PSUM") as pss:
            S0 = pss.tile([128, C], F32, tag="S0", name="S0")
            S1 = pss.tile([64, C], F32, tag="S1", name="S1")
            nc.tensor.matmul(S0[:], sqra[:], skr[:], start=True, stop=True)
            nc.tensor.matmul(S1[:], sqrb[:], skr[:], start=True, stop=True)
            for nm, G, S, mk, rows in (("a", G0, S0, sb["maska"], 128),
                                       ("b", G1, S1, sb["maskb"], 64)):
                ssb = smx.tile([rows, C], F32, name=f"ssb{nm}")
                nc.scalar.copy(ssb[:], S[:])
                lg = smx.tile([rows, C], F32, name=f"lg{nm}")
                nc.vector.tensor_tensor(out=lg[:], in0=G[:], in1=ssb[:], op=MULT)
                nc.vector.tensor_tensor(out=lg[:], in0=lg[:], in1=mk[:], op=ADD)
                mx = smx.tile([rows, 1], F32, name=f"mx{nm}")
                nc.vector.reduce_max(mx[:], lg[:], axis=AX)
                nc.vector.tensor_scalar_mul(mx[:], mx[:], -1.0)
                ssum = smx.tile([rows, 1], F32, name=f"ss{nm}")
                nc.scalar.activation(lg[:], lg[:], AF.Exp, bias=mx[:],
                                     accum_out=ssum[:])
                nc.vector.reciprocal(ssum[:], ssum[:])
                a16 = smx.tile([rows, C], F16, name=f"a16{nm}")
                nc.vector.tensor_scalar_mul(a16[:], lg[:], ssum[:])
                at[nm] = a16

        with tc.tile_pool(name="psm", bufs=1, space="PSUM") as psm:
            MT0 = psm.tile([128, C], F32, tag="MT0", name="MT0")
            MT1 = psm.tile([64, C], F32, tag="MT1", name="MT1")
            nc.tensor.matmul(MT0[:], at["a"][:, 0:128], sb["wpta"][:],
                             start=True, stop=False)
            nc.tensor.matmul(MT0[:], at["b"][:, 0:128], sb["wptb"][:],
                             start=False, stop=True)
            nc.tensor.matmul(MT1[:], at["a"][:, 128:C], sb["wpta"][:],
                             start=True, stop=False)
            nc.tensor.matmul(MT1[:], at["b"][:, 128:C], sb["wptb"][:],
                             start=False, stop=True)
            nc.vector.tensor_copy(mt1[:], MT0[0:64, :])
            nc.vector.tensor_copy(mt2[0:64, :], MT0[64:128, :])
            nc.vector.tensor_copy(mt2[64:128, :], MT1[:])

    # ================= phase C: O = (Wp @ A) @ vd =========================
    NG = HWTOT // 2048
    with tc.tile_pool(name="vst", bufs=3) as vst, \
         tc.tile_pool(name="ost", bufs=3) as ost, \
         tc.tile_pool(name="pso", bufs=2, space="PSUM") as pso:
        for g in range(NG):
            gsl = slice(g * 2048, (g + 1) * 2048)
            vA = vst.tile([64, 2048], F16, tag="vA", name="vA")
            nc.sync.dma_start(vA[:], io["vdram"][0:64, gsl])
            vB = vst.tile([128, 2048], F16, tag="vB", name="vB")
            nc.sync.dma_start(vB[:], io["vdram"][64:C, gsl])
            for hp in range(2):
                O0 = pso.tile([128, 1024], F32, tag="O0", name="O0")
                O1 = pso.tile([64, 1024], F32, tag="O1", name="O1")
                for t in range(2):
                    vsl = slice(hp * 1024 + t * 512, hp * 1024 + (t + 1) * 512)
                    osl = slice(t * 512, (t + 1) * 512)
                    nc.tensor.matmul(O0[:, osl], mt1[:, 0:128], vA[:, vsl],
                                     start=True, stop=False)
                    nc.tensor.matmul(O0[:, osl], mt2[:, 0:128], vB[:, vsl],
                                     start=False, stop=True)
                    nc.tensor.matmul(O1[:, osl], mt1[:, 128:C], vA[:, vsl],
                                     start=True, stop=False)
                    nc.tensor.matmul(O1[:, osl], mt2[:, 128:C], vB[:, vsl],
                                     start=False, stop=True)
                oa = ost.tile([128, 1024], F16, tag="oa", name="oa")
                ob = ost.tile([64, 1024], F16, tag="ob", name="ob")
                nc.scalar.copy(oa[:], O0[:])
                nc.vector.tensor_copy(ob[:], O1[:])
                psl = slice(g * 2048 + hp * 1024, g * 2048 + (hp + 1) * 1024)
                nc.scalar.dma_start(io["out"][0:128, psl], oa[:])
                nc.scalar.dma_start(io["out"][128:C, psl], ob[:])
    st.close()


def build_module():
    nc = bacc.Bacc("TRN2")
    io = {}
    io["kv"] = nc.dram_tensor("kv", [C, HWTOT], F16, kind="ExternalInput").ap()
    io["q"] = nc.dram_tensor("q", [C, HWTOT], F16, kind="ExternalInput").ap()
    io["w1t"] = nc.dram_tensor("w1t", [C, C2], F16, kind="ExternalInput").ap()
    io["w2d"] = nc.dram_tensor("w2d", [27, 128, 128], F16, kind="ExternalInput").ap()
    io["w2v"] = nc.dram_tensor("w2v", [128, 27], F32, kind="ExternalInput").ap()
    io["w2vn"] = nc.dram_tensor("w2vn", [128, 27], F32, kind="ExternalInput").ap()
    io["wpt"] = nc.dram_tensor("wpt", [C, C], F16, kind="ExternalInput").ap()
    io["mask"] = nc.dram_tensor("mask", [C, C], F32, kind="ExternalInput").ap()
    io["scale192"] = nc.dram_tensor("scale192", [C, 1], F32, kind="ExternalInput").ap()
    io["out"] = nc.dram_tensor("out", [C, HWTOT], F16, kind="ExternalOutput").ap()
    io["vdram"] = nc.dram_tensor("vdram", [C, HWTOT], F16).ap()
    io["nrm"] = nc.dram_tensor("nrm", [2, C], F32).ap()
    with tile.TileContext(nc) as tc:
        emit_kernel(tc, io)
    nc.compile()
    return nc


def prep_weights(qkv1_w, qkv2_w, proj_w, scale):
    w1 = np.asarray(qkv1_w).reshape(C2, C)
    w1t = np.ascontiguousarray(w1.T).astype(np.float16)
    w2 = np.asarray(qkv2_w).reshape(C2, 9)
    w2d = np.zeros((27, 128, 128), np.float16)
    for mc in range(3):
        for wi in range(9):
            np.fill_diagonal(w2d[mc * 9 + wi], w2[mc * 128:(mc + 1) * 128, wi])
    w2v = np.zeros((128, 27), np.float32)
    for mc in range(3):
        w2v[:, mc * 9:(mc + 1) * 9] = w2[mc * 128:(mc + 1) * 128, :]
    wpt = np.ascontiguousarray(np.asarray(proj_w).reshape(C, C).T).astype(np.float16)
    mask = np.full((C, C), -1e30, np.float32)
    for h in range(HEADS):
        mask[h * CD:(h + 1) * CD, h * CD:(h + 1) * CD] = 0.0
    scale192 = np.repeat(np.asarray(scale).reshape(HEADS), CD).astype(
        np.float32).reshape(C, 1)
    return {"w1t": w1t, "w2d": w2d, "w2v": w2v, "w2vn": -w2v, "wpt": wpt,
            "mask": mask, "scale192": scale192}


_CACHED = {}


def kernel(kv, q, qkv1_w, qkv2_w, proj_w, scale):
    kv = np.asarray(kv, np.float32)
    q = np.asarray(q, np.float32)
    b = kv.shape[0]
    assert b == 8 and kv.shape[1] == C
    wts = prep_weights(qkv1_w, qkv2_w, proj_w, scale)
    if "nc" not in _CACHED:
        nc = build_module()
        nc.m = get_hw_module(nc.m)
        _CACHED["nc"] = nc
    nc = _CACHED["nc"]
    in_maps = []
    for i in range(b):
        m = {"kv": np.ascontiguousarray(kv[i].reshape(C, HWTOT)).astype(np.float16),
             "q": np.ascontiguousarray(q[i].reshape(C, HWTOT)).astype(np.float16)}
        m.update(wts)
        in_maps.append(m)
    res = run_bass_kernel_spmd(nc, in_maps, core_ids=list(range(8)))
    out = np.stack([res.results[i]["out"].reshape(C, H, W) for i in range(b)])
    return out.astype(np.float32)
